# revision 4
# baseline (speedup 1.0000x reference)
"""Causal self-attention with RoPE on 8 Trainium2 NeuronCores.

Strategy: data-parallel over batch (B=8 -> 1 batch element per core), no
collectives. Per core, everything is computed in a transposed dataflow so no
large on-device transposes are needed:

  - host sends x^T (bf16) plus head-permuted W_qk so the QKV matmul directly
    produces Q^T/K^T in an even/odd-split row layout,
  - RoPE is applied in that layout with 4 partition-shift DMAs + 3 vector ops
    per 128-row tile against host-precomputed cos/sin tiles,
  - scores are computed already-transposed (S^T[k,q]) with two heads packed
    into the 128-wide PE array (contraction dim is HD=64),
  - softmax: exp on the scalar engine straight out of PSUM (scale=1/8 fused,
    no max-subtraction needed at these magnitudes); the denominator falls out
    of the AV matmul by appending a ones-column to V; normalization is folded
    into the PSUM->SBUF move,
  - the output projection uses Ycat^T as the stationary operand so y lands in
    natural [T, C] layout.
"""

import numpy as np
import ml_dtypes

import concourse.bass as bass
import concourse.tile as tile
from concourse import bacc, mybir
from concourse import bass_utils

F32 = mybir.dt.float32
BF16 = mybir.dt.bfloat16
AF = mybir.ActivationFunctionType

B, T, C = 8, 2048, 1024
H, HD = 16, 64
NCORES = 8

NT = T // 128      # 16 t-tiles
TCH = T // 512     # 4 t-chunks
CCH = C // 128     # 8 c-chunks
NJT = 16           # 8 Q + 8 K row-tiles of the [2C, T] QK^T output
NHP = H // 2       # 8 head pairs


def _build(nc, repeat=1, parts=('p1', 'p2', 'p3'), loop_n=0):
    """loop_n>0 wraps the body in an on-device For_i loop (for benchmarking)."""
    xt = nc.dram_tensor("xt", [C, T], BF16, kind="ExternalInput").ap()
    wqk = nc.dram_tensor("wqk", [C, 2 * C], BF16, kind="ExternalInput").ap()
    wv = nc.dram_tensor("wv", [C, C], BF16, kind="ExternalInput").ap()
    wproj = nc.dram_tensor("wproj", [C, C], BF16, kind="ExternalInput").ap()
    cos4 = nc.dram_tensor("cos4", [128, T], BF16, kind="ExternalInput").ap()
    sin4s = nc.dram_tensor("sin4s", [128, T], BF16, kind="ExternalInput").ap()
    maskd = nc.dram_tensor("maskd", [128, 128], BF16, kind="ExternalInput").ap()
    bqk = nc.dram_tensor("bqk", [128, NJT], F32, kind="ExternalInput").ap()
    bvb = nc.dram_tensor("bvb", [128, C], BF16, kind="ExternalInput").ap()
    bpb = nc.dram_tensor("bpb", [128, C], F32, kind="ExternalInput").ap()
    y = nc.dram_tensor("y", [T, C], F32, kind="ExternalOutput").ap()
    dscr = nc.dram_tensor("dscr", [NHP * 4, 1024], F32).ap()  # denom bounce

    from contextlib import ExitStack
    with tile.TileContext(nc) as tc, ExitStack() as ctx:
        ep = ctx.enter_context
        persist = ep(tc.tile_pool(name="persist", bufs=1))
        consts = ep(tc.tile_pool(name="consts", bufs=1))
        ph1 = ep(tc.tile_pool(name="ph1", bufs=1))
        wq_pool = ep(tc.tile_pool(name="wq_pool", bufs=3))
        qkp = ep(tc.tile_pool(name="qkp", bufs=2))
        rope_sw = ep(tc.tile_pool(name="rope_sw", bufs=2))
        rope_tmp = ep(tc.tile_pool(name="rope_tmp", bufs=1))
        pt_pool = ep(tc.tile_pool(name="pt_pool", bufs=3))
        nrm_pool = ep(tc.tile_pool(name="nrm", bufs=2))
        yo_pool = ep(tc.tile_pool(name="yo_pool", bufs=2))
        wbig = ep(tc.tile_pool(name="wbig", bufs=1))
        ps_s = ep(tc.tile_pool(name="ps_s", bufs=2, space="PSUM"))
        ps_y = ep(tc.tile_pool(name="ps_y", bufs=2, space="PSUM"))

        cos_s = consts.tile([128, T], BF16)
        sin_s = consts.tile([128, T], BF16)
        mask_s = consts.tile([128, 128], BF16)
        bqk_s = consts.tile([128, NJT], F32)
        bvb_s = consts.tile([128, C], BF16)
        bpb_s = consts.tile([128, C], F32)
        nc.sync.dma_start(out=cos_s, in_=cos4)
        nc.sync.dma_start(out=sin_s, in_=sin4s)
        nc.sync.dma_start(out=mask_s, in_=maskd)
        nc.sync.dma_start(out=bqk_s, in_=bqk)
        nc.sync.dma_start(out=bvb_s, in_=bvb)
        nc.sync.dma_start(out=bpb_s, in_=bpb)

        def qk_slot(hp, xts, which):
            """Compute one 128-row tile of Q^T or K^T (2 heads) + RoPE."""
            jt = hp if which == 0 else 8 + hp
            dst = qkp.tile([128, T], BF16, tag="qk" + str(which),
                           name=f"qk{which}_{hp}")
            wt = wq_pool.tile([128, CCH, 128], BF16, tag="wqk", name=f"wt{jt}")
            nc.sync.dma_start(
                out=wt,
                in_=bass.AP(tensor=wqk.tensor, offset=wqk.offset + 128 * jt,
                            ap=[[2 * C, 128], [128 * 2 * C, CCH], [1, 128]]))
            for tck2 in range(TCH // 2):
                ps = ps_s.tile([128, 1024], F32, tag="pss", name=f"psqk{tck2}")
                for half in range(2):
                    t0 = 1024 * tck2 + 512 * half
                    for ci in range(CCH):
                        nc.tensor.matmul(
                            ps[:, 512 * half:512 * (half + 1)],
                            wt[:, ci, :], xts[:, ci, t0:t0 + 512],
                            start=(ci == 0), stop=(ci == CCH - 1))
                nc.vector.tensor_scalar_add(
                    dst[:, 1024 * tck2:1024 * (tck2 + 1)], ps,
                    bqk_s[:, jt:jt + 1])
            sw = rope_sw.tile([128, T], BF16, tag="sw", name=f"sw{jt}")
            nc.sync.dma_start(out=sw[0:32, :], in_=dst[32:64, :])
            nc.sync.dma_start(out=sw[32:64, :], in_=dst[0:32, :])
            nc.sync.dma_start(out=sw[64:96, :], in_=dst[96:128, :])
            nc.sync.dma_start(out=sw[96:128, :], in_=dst[64:96, :])
            tmp = rope_tmp.tile([128, T], BF16, tag="tmp", name=f"tmp{jt}")
            nc.vector.tensor_mul(tmp, dst, cos_s)
            nc.vector.tensor_mul(sw, sw, sin_s)
            nc.vector.tensor_add(dst, tmp, sw)
            return dst

        def attention(hp, qtile, ktile, vs, yc):
            for qc2 in range(2):
                kmax = 8 * (qc2 + 1)
                for h in range(2):
                    pr = slice(64 * h, 64 * h + 64)
                    psY = ps_y.tile([HD + 1, 1024], F32, tag="psy",
                                    name=f"psY{h}")
                    for kti in range(kmax):
                        g = kti - 8 * qc2
                        qlo = max(0, 128 * g)
                        ks = slice(128 * kti, 128 * (kti + 1))
                        psS = ps_s.tile([128, 1024], F32, tag="pss",
                                        name=f"psS{kti}")
                        pt = pt_pool.tile([128, 1024], BF16, tag="pt",
                                          name=f"pt{kti}")
                        for sh in range(2):
                            slo = max(qlo, 512 * sh)
                            shi = 512 * (sh + 1)
                            if slo >= shi:
                                continue
                            nc.tensor.matmul(
                                psS[:, slo:shi], ktile[pr, ks],
                                qtile[pr, 1024 * qc2 + slo:1024 * qc2 + shi],
                                start=True, stop=True)
                        nc.scalar.activation(
                            pt[:, qlo:1024], psS[:, qlo:1024],
                            AF.Exp, scale=0.125)
                        if g >= 0:
                            nc.vector.tensor_mul(
                                pt[:, qlo:qlo + 128],
                                pt[:, qlo:qlo + 128], mask_s)
                        for h2 in range(2):
                            lo = max(qlo, 512 * h2)
                            hi = 512 * (h2 + 1)
                            if lo >= hi:
                                continue
                            nc.tensor.matmul(
                                psY[:, lo:hi],
                                vs[kti][:, 2 * hp + h, :], pt[:, lo:hi],
                                start=(kti == 0), stop=(kti == kmax - 1),
                                skip_group_check=True)
                    i = hp * 4 + qc2 * 2 + h
                    rec = nrm_pool.tile([1, 1024], F32, tag="rec",
                                        name=f"rec{h}")
                    nc.vector.reciprocal(rec, psY[HD:HD + 1, :])
                    nc.sync.dma_start(out=dscr[i:i + 1, :], in_=rec)
                    rb = nrm_pool.tile([64, 1024], F32, tag="rb",
                                       name=f"rb{h}")
                    nc.sync.dma_start(
                        out=rb, in_=dscr[i:i + 1, :].partition_broadcast(64))
                    nc.vector.tensor_mul(
                        yc[hp][64 * h:64 * h + 64,
                               1024 * qc2:1024 * (qc2 + 1)],
                        psY[0:HD, :], rb)

        def body():
            vs = [persist.tile([128, H, HD + 1], BF16, tag=f"vs{v}",
                               name=f"vs{v}") for v in range(NT)]
            yc = [persist.tile([128, T], BF16, tag=f"yc{s}", name=f"yc{s}")
                  for s in range(NHP)]
            xts = ph1.tile([128, CCH, T], BF16, tag="xts", name="xts")
            if 'p1' in parts:
                for ci in range(CCH):
                    nc.sync.dma_start(
                        out=xts[:, ci, :], in_=xt[128 * ci:128 * (ci + 1), :])
                for v in range(NT):
                    nc.vector.memset(vs[v][:, :, HD:HD + 1], 1.0)
                wvt = wbig.tile([128, CCH, 2, 512], BF16, tag="wbig",
                                name="wvt")
                for ci in range(CCH):
                    nc.sync.dma_start(
                        out=wvt[:, ci, :, :],
                        in_=bass.AP(tensor=wv.tensor,
                                    offset=wv.offset + 128 * ci * C,
                                    ap=[[C, 128], [512, 2], [1, 512]]))
                for vt in range(NT):
                    ps = ps_s.tile([128, 1024], F32, tag="pss", name=f"psv{vt}")
                    for nck in range(2):
                        for ci in range(CCH):
                            nc.tensor.matmul(
                                ps[:, 512 * nck:512 * (nck + 1)],
                                xts[:, ci, 128 * vt:128 * (vt + 1)],
                                wvt[:, ci, nck, :],
                                start=(ci == 0), stop=(ci == CCH - 1))
                    nc.vector.tensor_add(vs[vt][:, :, 0:HD], ps, bvb_s)

            for hp in range(NHP):
                if 'p1' in parts:
                    qtile = qk_slot(hp, xts, 0)
                    ktile = qk_slot(hp, xts, 1)
                else:
                    qtile = qkp.tile([128, T], BF16, tag="qk0", name="qk0d")
                    ktile = qkp.tile([128, T], BF16, tag="qk1", name="qk1d")
                    nc.vector.memset(qtile[:, 0:8], 0.0)
                    nc.vector.memset(ktile[:, 0:8], 0.0)
                if 'p2' in parts:
                    attention(hp, qtile, ktile, vs, yc)

            if 'p2' not in parts:
                for s in range(NHP):
                    nc.vector.memset(yc[s][:, 0:8], 0.0)
            if 'p1' not in parts:
                for v in range(NT):
                    nc.vector.memset(vs[v][:, 0, 0:8], 0.0)

            # ---------------- output projection ------------------------
            if 'p3' in parts:
                wpt = wbig.tile([128, CCH, 2, 512], BF16, tag="wbig",
                                name="wpt")
                for ci in range(CCH):
                    nc.sync.dma_start(
                        out=wpt[:, ci, :, :],
                        in_=bass.AP(tensor=wproj.tensor,
                                    offset=wproj.offset + 128 * ci * C,
                                    ap=[[C, 128], [512, 2], [1, 512]]))
                for tt in range(NT):
                    ps = ps_s.tile([128, 1024], F32, tag="pss", name=f"psp{tt}")
                    for ec in range(2):
                        for ci in range(CCH):
                            nc.tensor.matmul(
                                ps[:, 512 * ec:512 * (ec + 1)],
                                yc[ci][:, 128 * tt:128 * (tt + 1)],
                                wpt[:, ci, ec, :],
                                start=(ci == 0), stop=(ci == CCH - 1))
                    yo = yo_pool.tile([128, 1024], F32, tag="yo",
                                      name=f"yo{tt}")
                    nc.vector.tensor_add(yo, ps, bpb_s)
                    nc.sync.dma_start(
                        out=y[128 * tt:128 * (tt + 1), :], in_=yo)

        if loop_n > 0:
            with tc.For_i(0, loop_n, 1) as _i:
                body()
        else:
            for _rep in range(repeat):
                body()
    return nc


_PERM = None


def _head_perm():
    """Column permutation for W_qk: within each head, evens then odds."""
    global _PERM
    if _PERM is None:
        within = np.concatenate([np.arange(0, HD, 2), np.arange(1, HD, 2)])
        _PERM = (np.arange(H)[:, None] * HD + within[None, :]).reshape(-1)
    return _PERM


def _prep_shared(freqs, W_attn, b_attn, W_proj, b_proj):
    bf = ml_dtypes.bfloat16
    perm = _head_perm()
    wq = W_attn[:, 0:C][:, perm]
    wk = W_attn[:, C:2 * C][:, perm]
    wqk = np.concatenate([wq, wk], axis=1).astype(bf)
    wv = np.ascontiguousarray(W_attn[:, 2 * C:3 * C]).astype(bf)
    wproj = np.ascontiguousarray(W_proj).astype(bf)

    cos = np.cos(freqs.astype(np.float64)).astype(np.float32)   # [T, 32]
    sin = np.sin(freqs.astype(np.float64)).astype(np.float32)
    cos4 = np.empty((128, T), np.float32)
    sin4s = np.empty((128, T), np.float32)
    for blk in range(4):
        cos4[32 * blk:32 * blk + 32] = cos.T
        sgn = -1.0 if blk % 2 == 0 else 1.0
        sin4s[32 * blk:32 * blk + 32] = sgn * sin.T
    maskd = (np.arange(128)[:, None] <= np.arange(128)[None, :]).astype(np.float32)

    bq = b_attn[0:C][perm]
    bk = b_attn[C:2 * C][perm]
    bqk = np.concatenate([bq, bk]).reshape(NJT, 128).T.astype(np.float32)
    bqk = np.ascontiguousarray(bqk)
    bvb = np.broadcast_to(b_attn[2 * C:3 * C], (128, C)).astype(np.float32)
    bpb = np.broadcast_to(b_proj, (128, C)).astype(np.float32)
    return {
        "wqk": wqk, "wv": wv, "wproj": wproj,
        "cos4": cos4.astype(bf), "sin4s": sin4s.astype(bf),
        "maskd": maskd.astype(bf), "bqk": bqk,
        "bvb": np.ascontiguousarray(bvb).astype(bf), "bpb": np.ascontiguousarray(bpb),
    }


_CACHE = {}


def _get_nc():
    if "nc" not in _CACHE:
        nc = bacc.Bacc("TRN2", target_bir_lowering=False, debug=False,
                       num_devices=NCORES)
        _build(nc)
        nc.compile()
        _CACHE["nc"] = nc
    return _CACHE["nc"]


def kernel(x, freqs, W_attn, b_attn, W_proj, b_proj, **_unused):
    x = np.asarray(x, dtype=np.float32)
    shared = _prep_shared(
        np.asarray(freqs, np.float32), np.asarray(W_attn, np.float32),
        np.asarray(b_attn, np.float32), np.asarray(W_proj, np.float32),
        np.asarray(b_proj, np.float32))
    bf = ml_dtypes.bfloat16
    in_maps = []
    for b in range(NCORES):
        xtb = np.ascontiguousarray(x[b].T).astype(bf)    # [C, T]
        in_maps.append({"xt": xtb, **shared})

    nc = _get_nc()
    res = bass_utils.run_bass_kernel_spmd(nc, in_maps, core_ids=list(range(NCORES)))
    out = np.stack([res.results[b]["y"] for b in range(NCORES)], axis=0)
    return out.astype(np.float32)

